# revision 1
# baseline (speedup 1.0000x reference)
"""Trainium2 Bass kernel for a 4-layer dense transformer encoder.

Problem: nn_Encoder (LAYERS=4, D_MODEL=1024, HEADS=16, HIDDEN=4096, B=2, L=2048).

Sharding: the 4096 (batch, token) rows are split into 8 chunks of 512 tokens —
cores 0-3 take batch 0, cores 4-7 take batch 1. Weights are replicated. Per
layer each core projects Q/K/V for its own 512 tokens; the K/V shards are
all-gathered within each 4-core group in TWO halves (tokens 0:256 and 256:512)
so the second gather overlaps the first half's attention, and the first
overlaps the remaining projections. Each core then runs full attention for its
512 queries over all 2048 keys, followed by the output projection and FFN.

On-chip layout: activations are feature-major [D, tok] so every linear is a
chain of 128x128-stationary weight-tile matmuls streaming the activation
[128, 512]. V is produced token-major (activation stationary, weight streamed)
because the context matmul needs V[token, dk]; a constant 1.0 column is
appended per head so the ctx matmul chain also produces the softmax
denominator in PSUM row 64. Scores for a head PAIR run as two concurrent
row-tiled matmuls (contract=64 at partition bases 0 and 64) into one 2-bank
PSUM tile, giving one [128, 1024] exp per (pair, key-tile). Softmax runs
without max-subtraction (scores are well-scaled: weights ~N(0, 0.02^2)).
Matmul inputs are bf16 (fp32 accumulate); the residual stream stays fp32.
"""

import numpy as np
import ml_dtypes

import concourse.bass as bass
import concourse.mybir as mybir
import concourse.tile as tile
from concourse import bacc
from concourse.bass_utils import run_bass_kernel_spmd

F32 = mybir.dt.float32
BF16 = mybir.dt.bfloat16
AF = mybir.ActivationFunctionType

LAYERS, D, HEADS, DK, HID = 4, 1024, 16, 64, 4096
B, L = 2, 2048
P = 128
TOK = 512          # tokens per core
HTOK = 256         # tokens per gather half
FT = D // P        # 8 feature tiles
HT = HID // P      # 32 hidden tiles
RANKS = 4          # cores per gather group
VE = DK + 1        # 65: per-head V columns + ones column
KSZ = FT * P * HTOK          # K staging elems per half (feature-major [8,128,256])
VSZ = HTOK * HEADS * VE      # V staging elems per half (token-major [256,16,65])
CH = KSZ + VSZ               # combined staging elems per rank per half
N_CORES = 8

_CACHE = {}


def build_nc(layers=LAYERS):
    nc = bacc.Bacc("TRN2", target_bir_lowering=False, debug=False,
                   num_devices=N_CORES)
    LY = layers
    x_fm = nc.dram_tensor("x_fm", [FT, P, TOK], F32, kind="ExternalInput").ap()
    wqk = nc.dram_tensor("wqk", [LY, 16, P, FT, P], BF16, kind="ExternalInput").ap()
    wv = nc.dram_tensor("wv", [LY, P, FT, D], BF16, kind="ExternalInput").ap()
    wout = nc.dram_tensor("wout", [LY, FT, P, FT, P], BF16, kind="ExternalInput").ap()
    w1 = nc.dram_tensor("w1", [LY, HT, P, FT, P], BF16, kind="ExternalInput").ap()
    w2 = nc.dram_tensor("w2", [LY, FT, P, HT, P], BF16, kind="ExternalInput").ap()
    bqk = nc.dram_tensor("bqk", [LY, P, 16], F32, kind="ExternalInput").ap()
    bv = nc.dram_tensor("bv", [LY, 1, D], BF16, kind="ExternalInput").ap()
    bout = nc.dram_tensor("bout", [LY, P, FT], F32, kind="ExternalInput").ap()
    b1 = nc.dram_tensor("b1", [LY, P, HT], F32, kind="ExternalInput").ap()
    b2 = nc.dram_tensor("b2", [LY, P, FT], F32, kind="ExternalInput").ap()
    out_fm = nc.dram_tensor("out_fm", [FT, P, TOK], F32, kind="ExternalOutput").ap()

    groups = [[0, 1, 2, 3], [4, 5, 6, 7]]

    from contextlib import ExitStack

    with tile.TileContext(nc) as tc:
        with ExitStack() as stack:
            pool = lambda name, bufs, **kw: stack.enter_context(
                tc.tile_pool(name=name, bufs=bufs, **kw))
            hp = pool("hp", 2)
            hbfp = pool("hbfp", 2)
            qp = pool("qp", 1)
            khp = pool("khp", 2)
            vallp = pool("vallp", 2)
            ep = pool("ep", 4)
            wkp = pool("wkp", 3)
            wkkp = pool("wkkp", 8)
            w2p = pool("w2p", 3)
            wvp = pool("wvp", 1)
            gp = pool("gp", 1)
            cbfp = pool("cbfp", 1)
            kvp = pool("kvp", 2)
            t1p = pool("t1p", 3)
            biasp = pool("biasp", 1)
            smallp = pool("smallp", 1)
            constp = pool("constp", 1)
            plinp = pool("plinp", 2, space="PSUM")
            psp = pool("psp", 2, space="PSUM")
            pctxp = pool("pctxp", 2, space="PSUM")
            dramp = pool("dramp", 2, space="DRAM")

            ones_bf = constp.tile([1, P], BF16)
            nc.vector.memset(ones_bf[:], 1.0)
            ones_f32 = constp.tile([1, P], F32)
            nc.vector.memset(ones_f32[:], 1.0)

            h = hp.tile([P, FT, TOK], F32, tag="h", name="h0")
            nc.sync.dma_start(h[:], x_fm.rearrange("a p t -> p a t"))

            def kv_project_half(ly, half, h_bf, wv_sb, bqk_sb, bv_sb, wkts):
                """K (feature-major) + V (token-major, +ones +bias row) for one
                token half -> DRAM staging; returns the staging tile."""
                t0 = half * HTOK
                kv_stage = dramp.tile([CH], BF16, tag=f"stage{half}",
                                      name=f"stage_{ly}_{half}")
                k_view = kv_stage[0:KSZ].rearrange("(a p t) -> a p t", p=P, t=HTOK)
                v_view = kv_stage[KSZ:CH].rearrange("(t h e) -> t h e",
                                                    h=HEADS, e=VE)
                for nt in range(FT):
                    pp = plinp.tile([P, TOK], F32, tag="plin",
                                    name=f"ppk_{ly}_{half}_{nt}")
                    for kt in range(FT):
                        nc.tensor.matmul(pp[:, 0:HTOK], lhsT=wkts[nt][:, kt, :],
                                         rhs=h_bf[:, kt, t0:t0 + HTOK],
                                         start=(kt == 0), stop=(kt == FT - 1))
                    k_sb = kvp.tile([P, HTOK], BF16, tag="ksb",
                                    name=f"ksb_{ly}_{half}_{nt}")
                    nc.vector.tensor_scalar_add(k_sb[:], pp[:, 0:HTOK],
                                                bqk_sb[:, FT + nt:FT + nt + 1])
                    nc.sync.dma_start(k_view[nt], k_sb[:])
                for tt in range(2):
                    for ch in range(2):
                        pp = plinp.tile([P, TOK], F32, tag="plin",
                                        name=f"ppv_{ly}_{half}_{tt}_{ch}")
                        for kt in range(FT):
                            nc.tensor.matmul(
                                pp[:], lhsT=h_bf[:, kt, t0 + tt * P:t0 + tt * P + P],
                                rhs=wv_sb[:, kt, ch * 512:(ch + 1) * 512],
                                start=(kt == 0), stop=False)
                        nc.tensor.matmul(pp[:], lhsT=ones_bf[:],
                                         rhs=bv_sb[:, ch * 512:(ch + 1) * 512],
                                         start=False, stop=True)
                        v_sb = kvp.tile([P, FT, VE], BF16, tag="vsb",
                                        name=f"vsb_{ly}_{half}_{tt}_{ch}")
                        nc.scalar.activation(
                            v_sb[:, :, 0:DK],
                            pp[:].rearrange("p (a b) -> p a b", b=DK), AF.Copy)
                        nc.vector.memset(v_sb[:, :, DK:VE], 1.0)
                        nc.sync.dma_start(
                            v_view[tt * P:(tt + 1) * P, ch * FT:(ch + 1) * FT, :],
                            v_sb[:])
                return kv_stage

            def all_gather(ly, half, kv_stage):
                kv_gath = dramp.tile([RANKS * CH], BF16, tag=f"gath{half}",
                                     name=f"gath_{ly}_{half}")
                nc.gpsimd.collective_compute(
                    "AllGather", mybir.AluOpType.bypass, replica_groups=groups,
                    ins=[kv_stage.opt()], outs=[kv_gath.opt()])
                return kv_gath

            for ly in range(layers):
                # ---- per-layer weights/biases ----
                wv_sb = wvp.tile([P, FT, D], BF16, tag="wv", name=f"wv_{ly}")
                nc.sync.dma_start(wv_sb[:], wv[ly])
                bqk_sb = biasp.tile([P, 16], F32, tag="bqk", name=f"bqk_{ly}")
                nc.sync.dma_start(bqk_sb[:], bqk[ly])
                bv_sb = biasp.tile([1, D], BF16, tag="bv", name=f"bv_{ly}")
                nc.sync.dma_start(bv_sb[:], bv[ly])
                bout_sb = biasp.tile([P, FT], F32, tag="bout", name=f"bout_{ly}")
                nc.sync.dma_start(bout_sb[:], bout[ly])
                b1_sb = biasp.tile([P, HT], F32, tag="b1", name=f"b1_{ly}")
                nc.sync.dma_start(b1_sb[:], b1[ly])
                b2_sb = biasp.tile([P, FT], F32, tag="b2", name=f"b2_{ly}")
                nc.sync.dma_start(b2_sb[:], b2[ly])

                h_bf = hbfp.tile([P, FT, TOK], BF16, tag="hbf", name=f"hbf_{ly}")
                nc.vector.tensor_copy(out=h_bf[:], in_=h[:])

                # K-weight tiles are shared by both halves
                wkts = []
                for nt in range(FT):
                    wt = wkkp.tile([P, FT, P], BF16, tag="wkk", name=f"wk_k{ly}_{nt}")
                    nc.sync.dma_start(wt[:], wqk[ly, FT + nt])
                    wkts.append(wt)

                stage_a = kv_project_half(ly, 0, h_bf, wv_sb, bqk_sb, bv_sb, wkts)
                gath_a = all_gather(ly, 0, stage_a)
                stage_b = kv_project_half(ly, 1, h_bf, wv_sb, bqk_sb, bv_sb, wkts)
                gath_b = all_gather(ly, 1, stage_b)

                # ---- Q projection (PE work while gathers are in flight) ----
                q_sb = qp.tile([P, FT, TOK], BF16, tag="q", name=f"q_{ly}")
                for nt in range(FT):
                    wt = wkp.tile([P, FT, P], BF16, tag="wk", name=f"wk_q{ly}_{nt}")
                    nc.sync.dma_start(wt[:], wqk[ly, nt])
                    pp = plinp.tile([P, TOK], F32, tag="plin", name=f"ppq_{ly}_{nt}")
                    for kt in range(FT):
                        nc.tensor.matmul(pp[:], lhsT=wt[:, kt, :], rhs=h_bf[:, kt, :],
                                         start=(kt == 0), stop=(kt == FT - 1))
                    nc.vector.tensor_scalar_add(q_sb[:, nt, :], pp[:],
                                                bqk_sb[:, nt:nt + 1])

                # ---- attention: phase A over gathered half 0, then phase B ----
                ctx_bf = cbfp.tile([P, FT, TOK], BF16, tag="cbf", name=f"cbf_{ly}")
                h_mid = hp.tile([P, FT, TOK], F32, tag="h", name=f"hmid_{ly}")
                # phase-A partial ctx+denominator parked in DRAM, per head
                cacc_d = dramp.tile([HEADS, VE, TOK], F32, tag="cacc",
                                    name=f"cacc_{ly}")

                for half, gath in ((0, gath_a), (1, gath_b)):
                    rk = gath.rearrange("(r c) -> r c", c=CH)
                    v_all = vallp.tile([P, FT, HEADS * VE], BF16, tag="vall",
                                       name=f"vall_{ly}_{half}")
                    for r in range(RANKS):
                        src = rk[r, KSZ:CH].rearrange(
                            "(t he) -> t he", he=HEADS * VE
                        ).rearrange("(tt p) he -> p tt he", p=P)
                        nc.sync.dma_start(v_all[:, r * 2:(r + 1) * 2, :], src)

                    def finalize_pair(pcs_f, hpair_f):
                        if half == 0:
                            # park partial ctx+denom in DRAM
                            for sub in range(2):
                                hd = 2 * hpair_f + sub
                                tmp = t1p.tile([VE, TOK], F32, tag="t1",
                                               name=f"cpk_{ly}_{hd}")
                                nc.scalar.copy(out=tmp[:], in_=pcs_f[sub][:])
                                nc.sync.dma_start(cacc_d[hd], tmp[:])
                        else:
                            for sub in range(2):
                                hd = 2 * hpair_f + sub
                                base = sub * DK
                                ca = t1p.tile([VE, TOK], F32, tag="t1",
                                              name=f"ca_{ly}_{hd}")
                                nc.sync.dma_start(ca[:], cacc_d[hd])
                                c2 = t1p.tile([VE, TOK], F32, tag="t1",
                                              name=f"c2_{ly}_{hd}")
                                nc.vector.tensor_add(out=c2[:], in0=pcs_f[sub][:],
                                                     in1=ca[:])
                                dr = smallp.tile([1, TOK], F32, tag="dr",
                                                 name=f"dr_{ly}_{hd}")
                                nc.vector.reciprocal(dr[:], c2[DK:DK + 1, :])
                                drb = smallp.tile([1, TOK], BF16, tag="drb",
                                                  name=f"drb_{ly}_{hd}")
                                nc.vector.tensor_copy(out=drb[:], in_=dr[:])
                                pb = plinp.tile([P, TOK], F32, tag="plin",
                                                name=f"pb_{ly}_{hd}")
                                nc.tensor.matmul(pb[:], lhsT=ones_bf[:], rhs=drb[:],
                                                 start=True, stop=True)
                                nc.vector.tensor_tensor(
                                    out=ctx_bf[base:base + DK, hpair_f, :],
                                    in0=pb[0:DK, :], in1=c2[0:DK, :],
                                    op=mybir.AluOpType.mult)

                    def emit_scores(kh_s, hpair_s, kt):
                        r, ss = kt // 2, kt % 2
                        pS = psp.tile([P, 2, TOK], F32, tag="ps",
                                      name=f"ps_{ly}_{half}_{hpair_s}_{kt}")
                        for sub in range(2):
                            base = sub * DK
                            nc.tensor.matmul(
                                pS[:, sub, :],
                                lhsT=kh_s[base:base + DK, r, ss * P:(ss + 1) * P],
                                rhs=q_sb[base:base + DK, hpair_s, :],
                                start=True, stop=True)
                        return pS

                    # Per-pair processing is software-pipelined two ways: the
                    # next k-tile's scores are emitted before this k-tile's
                    # ctx matmuls (in-order PE keeps streaming while exp runs
                    # on ACT), and each pair's finalize (parking/normalize) is
                    # deferred until after the NEXT pair's first scores so its
                    # DVE chain + broadcast matmul don't stall the PE queue.
                    pending = None
                    for hpair in range(HEADS // 2):
                        kh = khp.tile([P, RANKS, HTOK], BF16, tag="kh",
                                      name=f"kh_{ly}_{half}_{hpair}")
                        ksrc = rk[:, hpair * (P * HTOK):(hpair + 1) * (P * HTOK)]
                        nc.sync.dma_start(
                            kh[:], ksrc.rearrange("r (p t) -> p r t", t=HTOK))
                        pcs = []
                        for sub in range(2):
                            pcs.append(pctxp.tile(
                                [VE, TOK], F32, tag="pctx",
                                name=f"pctx_{ly}_{half}_{2 * hpair + sub}"))
                        pS_cur = emit_scores(kh, hpair, 0)
                        if pending is not None:
                            finalize_pair(*pending)
                        for kt in range(FT):
                            pS_nxt = (emit_scores(kh, hpair, kt + 1)
                                      if kt + 1 < FT else None)
                            e_sb = ep.tile([P, 2, TOK], BF16, tag="e",
                                           name=f"e_{ly}_{half}_{hpair}_{kt}")
                            nc.scalar.activation(e_sb[:], pS_cur[:], AF.Exp,
                                                 scale=0.125)
                            for sub in range(2):
                                hd = 2 * hpair + sub
                                nc.tensor.matmul(
                                    pcs[sub][:],
                                    lhsT=v_all[:, kt, hd * VE:(hd + 1) * VE],
                                    rhs=e_sb[:, sub, :],
                                    start=(kt == 0), stop=(kt == FT - 1))
                            pS_cur = pS_nxt
                        pending = (pcs, hpair)
                    finalize_pair(*pending)

                # ---- output projection + residual ----
                hmid_bf = hbfp.tile([P, FT, TOK], BF16, tag="hbf",
                                    name=f"hmidbf_{ly}")
                for nt in range(FT):
                    wt = wkp.tile([P, FT, P], BF16, tag="wk", name=f"wk_o{ly}_{nt}")
                    nc.sync.dma_start(wt[:], wout[ly, nt])
                    pp = plinp.tile([P, TOK], F32, tag="plin", name=f"ppo_{ly}_{nt}")
                    for kt in range(FT):
                        nc.tensor.matmul(pp[:], lhsT=wt[:, kt, :],
                                         rhs=ctx_bf[:, kt, :],
                                         start=(kt == 0), stop=(kt == FT - 1))
                    t1 = t1p.tile([P, TOK], F32, tag="t1", name=f"t1o_{ly}_{nt}")
                    nc.vector.tensor_scalar_add(t1[:], pp[:], bout_sb[:, nt:nt + 1])
                    nc.vector.tensor_add(out=h_mid[:, nt, :], in0=t1[:],
                                         in1=h[:, nt, :])
                    nc.vector.tensor_copy(out=hmid_bf[:, nt, :], in_=h_mid[:, nt, :])

                # ---- FFN ----
                g = gp.tile([P, HT, TOK], BF16, tag="g", name=f"g_{ly}")
                for nt in range(HT):
                    wt = wkp.tile([P, FT, P], BF16, tag="wk", name=f"wk_f{ly}_{nt}")
                    nc.sync.dma_start(wt[:], w1[ly, nt])
                    pp = plinp.tile([P, TOK], F32, tag="plin", name=f"ppf_{ly}_{nt}")
                    for kt in range(FT):
                        nc.tensor.matmul(pp[:], lhsT=wt[:, kt, :],
                                         rhs=hmid_bf[:, kt, :],
                                         start=(kt == 0), stop=(kt == FT - 1))
                    nc.scalar.activation(g[:, nt, :], pp[:], AF.Gelu,
                                         bias=b1_sb[:, nt:nt + 1])

                h_out = hp.tile([P, FT, TOK], F32, tag="h", name=f"hout_{ly}")
                for nt in range(FT):
                    pp = plinp.tile([P, TOK], F32, tag="plin", name=f"ppf2_{ly}_{nt}")
                    for hh in range(2):
                        w2t = w2p.tile([P, HT // 2, P], BF16, tag="w2",
                                       name=f"w2_{ly}_{nt}_{hh}")
                        nc.sync.dma_start(w2t[:], w2[ly, nt, :, hh * 16:(hh + 1) * 16, :])
                        for k2 in range(HT // 2):
                            kt = hh * 16 + k2
                            nc.tensor.matmul(pp[:], lhsT=w2t[:, k2, :],
                                             rhs=g[:, kt, :],
                                             start=(kt == 0), stop=(kt == HT - 1))
                    t1 = t1p.tile([P, TOK], F32, tag="t1", name=f"t1f_{ly}_{nt}")
                    nc.vector.tensor_scalar_add(t1[:], pp[:], b2_sb[:, nt:nt + 1])
                    nc.vector.tensor_add(out=h_out[:, nt, :], in0=t1[:],
                                         in1=h_mid[:, nt, :])
                h = h_out

            nc.sync.dma_start(out_fm.rearrange("a p t -> p a t"), h[:])
    nc.compile()
    return nc


def _prep_inputs(x, Wqkv, bqkv, Wout, bout, W1, b1, W2, b2, layers=LAYERS):
    """Host-side re-tiling of the full inputs into per-core in_maps."""
    bf = ml_dtypes.bfloat16
    x = np.asarray(x, dtype=np.float32)
    Wqkv = np.asarray(Wqkv, dtype=np.float32)
    bqkv = np.asarray(bqkv, dtype=np.float32)
    Wout_ = np.asarray(Wout, dtype=np.float32)
    bout_ = np.asarray(bout, dtype=np.float32)
    W1_ = np.asarray(W1, dtype=np.float32)
    b1_ = np.asarray(b1, dtype=np.float32)
    W2_ = np.asarray(W2, dtype=np.float32)
    b2_ = np.asarray(b2, dtype=np.float32)
    LY = layers

    wqk = np.ascontiguousarray(
        Wqkv[:LY, :, :2 * D].reshape(LY, FT, P, 16, P).transpose(0, 3, 2, 1, 4)
    ).astype(bf)
    wv = np.ascontiguousarray(
        Wqkv[:LY, :, 2 * D:].reshape(LY, FT, P, D).transpose(0, 2, 1, 3)
    ).astype(bf)
    wout = np.ascontiguousarray(
        Wout_[:LY].reshape(LY, FT, P, FT, P).transpose(0, 3, 2, 1, 4)
    ).astype(bf)
    w1 = np.ascontiguousarray(
        W1_[:LY].reshape(LY, FT, P, HT, P).transpose(0, 3, 2, 1, 4)
    ).astype(bf)
    w2 = np.ascontiguousarray(
        W2_[:LY].reshape(LY, HT, P, FT, P).transpose(0, 3, 2, 1, 4)
    ).astype(bf)
    bqk = np.ascontiguousarray(
        bqkv[:LY, :2 * D].reshape(LY, 16, P).transpose(0, 2, 1))
    bvv = bqkv[:LY, 2 * D:].reshape(LY, 1, D).astype(bf)
    boutt = np.ascontiguousarray(bout_[:LY].reshape(LY, FT, P).transpose(0, 2, 1))
    b1t = np.ascontiguousarray(b1_[:LY].reshape(LY, HT, P).transpose(0, 2, 1))
    b2t = np.ascontiguousarray(b2_[:LY].reshape(LY, FT, P).transpose(0, 2, 1))

    shared = dict(wqk=wqk, wv=wv, wout=wout, w1=w1, w2=w2, bqk=bqk, bv=bvv,
                  bout=boutt, b1=b1t, b2=b2t)
    in_maps = []
    for c in range(N_CORES):
        b, r = divmod(c, RANKS)
        xc = x[b, r * TOK:(r + 1) * TOK, :]          # [512, 1024]
        x_fm = np.ascontiguousarray(xc.T).reshape(FT, P, TOK)
        in_maps.append({"x_fm": x_fm, **shared})
    return in_maps


def kernel(x, Wqkv, bqkv, Wout, bout, W1, b1, W2, b2):
    if "nc" not in _CACHE:
        _CACHE["nc"] = build_nc()
    nc = _CACHE["nc"]
    in_maps = _prep_inputs(x, Wqkv, bqkv, Wout, bout, W1, b1, W2, b2)
    res = run_bass_kernel_spmd(nc, in_maps, core_ids=list(range(N_CORES)))
    out = np.empty((B, L, D), dtype=np.float32)
    for c in range(N_CORES):
        b, r = divmod(c, RANKS)
        o = res.results[c]["out_fm"].reshape(D, TOK)      # [1024, 512]
        out[b, r * TOK:(r + 1) * TOK, :] = o.T
    return out

